# revision 5
# baseline (speedup 1.0000x reference)
"""Causal self-attention (B=2, S=2048, E=2048, H=16, rope) on 8 TRN2 NeuronCores.

Sharding: tensor-parallel over heads. Each core owns 2 heads (both batches):
  - w_qkv rows for its heads (q/k/v, 768 rows), w_out columns for its heads
    (256 cols). Every core reads the full x (replicated) and produces a
    partial [B*S, E] output; the host sums the 8 partials (the "all-reduce").

Per-core kernel layout choices:
  - x is fed pre-transposed as xT [E, B*S] in bf16 so it serves directly as
    the matmul rhs (Q/K projections, out rows = w rows -> QT/KT arrive
    transposed [D, S], which is exactly the layout attention needs) and as
    lhsT (V projection, natural [S, D] layout).
  - scores are computed transposed: scoresT[k,q] = KT^T @ QT. Softmax sums
    over k (the partition dim) are done with ones-vector matmuls on the PE;
    1/sum is broadcast across partitions with a K=1 matmul and folded into
    the PSUM->SBUF evacuation of y^T, so no per-element softmax division
    pass exists.
  - attn^T (bf16, zero-padded in the causally-invalid region) feeds A@V as
    lhsT with no transposes anywhere.
  - rope is applied on DVE while evacuating the QKV-projection PSUM, using
    host-precomputed cos / signed-sin tables in [D, S] layout; the softmax
    scale 1/sqrt(D) is folded into the exp activation's scale.
"""

import math

import numpy as np
import ml_dtypes

import concourse.bass as bass
import concourse.mybir as mybir
import concourse.tile as tile
from concourse import bacc
from concourse.bass_utils import run_bass_kernel_spmd

B, S, E, H, D = 2, 2048, 2048, 16, 128
NCORES = 8
HL = H // NCORES            # heads per core = 2
NTOK = B * S                # 4096
KE = E // 128               # 16 contraction chunks
NB = S // 128               # 16 k/token blocks per batch
NPANEL = S // 512           # 4 q panels per batch
SOFTMAX_SCALE = 1.0 / math.sqrt(D)
BF16 = mybir.dt.bfloat16
F32 = mybir.dt.float32

ROPE_BASE = 10000.0


def _rope_tables():
    inv_freq = 1.0 / (ROPE_BASE ** (np.arange(0, D, 2, dtype=np.float32) / D))
    pos = np.arange(S, dtype=np.float32)
    freqs = np.outer(pos, inv_freq)               # [S, D/2]
    emb = np.concatenate([freqs, freqs], -1)      # [S, D]
    cosT = np.cos(emb).T.astype(np.float32)       # [D, S]
    sinT = np.sin(emb).T.astype(np.float32)
    sinS = sinT.copy()
    sinS[: D // 2] *= -1.0                        # signed: rotate_half folded in
    return np.ascontiguousarray(cosT), np.ascontiguousarray(sinS)


def _emit(nc, tc, xT, wqkvT, w_outT, out, cos_d, sin_d, mask_d):
    from contextlib import ExitStack

    ctx = ExitStack()
    with ctx:
        singles = ctx.enter_context(tc.tile_pool(name="singles", bufs=1))
        xpool = ctx.enter_context(tc.tile_pool(name="xcol", bufs=2))
        persist = ctx.enter_context(tc.tile_pool(name="persist", bufs=1))
        ropet = ctx.enter_context(tc.tile_pool(name="ropet", bufs=3))
        attnp = ctx.enter_context(tc.tile_pool(name="attn", bufs=20))
        rowp = ctx.enter_context(tc.tile_pool(name="rowp", bufs=3))
        outp = ctx.enter_context(tc.tile_pool(name="outp", bufs=4))
        psum = ctx.enter_context(tc.tile_pool(name="psum", bufs=6, space="PSUM"))

        # ---- constants ----
        wq_sb = singles.tile([128, KE, 3 * HL * D], BF16, tag="wq")
        for ke in range(KE):
            nc.sync.dma_start(out=wq_sb[:, ke, :], in_=wqkvT[ke * 128:(ke + 1) * 128, :])
        wo_sb = singles.tile([128, HL, E], BF16, tag="wo")
        for hl in range(HL):
            nc.sync.dma_start(out=wo_sb[:, hl, :], in_=w_outT[hl * 128:(hl + 1) * 128, :])
        cos_sb = singles.tile([128, S], F32, tag="cos")
        sin_sb = singles.tile([128, S], F32, tag="sin")
        nc.sync.dma_start(out=cos_sb, in_=cos_d)
        nc.sync.dma_start(out=sin_sb, in_=sin_d)
        mask_sb = singles.tile([128, 128], BF16, tag="mask")
        nc.sync.dma_start(out=mask_sb, in_=mask_d)
        ones_k = singles.tile([128, 1], BF16, tag="onesk")
        nc.vector.memset(ones_k, 1.0)
        ones_1 = singles.tile([1, 128], F32, tag="ones1")
        nc.vector.memset(ones_1, 1.0)

        # ---- persistent per-(b,h) tensors ----
        q_sb = [[persist.tile([128, S], BF16, tag=f"q{b}{h}", name=f"q{b}{h}") for h in range(HL)] for b in range(B)]
        k_sb = [[persist.tile([128, S], BF16, tag=f"k{b}{h}", name=f"k{b}{h}") for h in range(HL)] for b in range(B)]
        v_sb = [persist.tile([128, NB, HL * D], BF16, tag=f"v{b}", name=f"v{b}") for b in range(B)]
        y_sb = [[persist.tile([128, S], BF16, tag=f"y{b}{h}", name=f"y{b}{h}") for h in range(HL)] for b in range(B)]

        # ================= phase 1: QKV projection + rope =================
        for tb in range(NTOK // 512):          # 8 token column-blocks of 512
            b = tb // (S // 512)
            soff = (tb % (S // 512)) * 512     # s-offset within batch
            xcol = xpool.tile([128, KE, 512], BF16, tag="xcol")
            for ke in range(KE):
                nc.sync.dma_start(
                    out=xcol[:, ke, :],
                    in_=xT[ke * 128:(ke + 1) * 128, tb * 512:(tb + 1) * 512],
                )
            # Q/K rows: transposed layout [D, s]
            for rb in range(2 * HL):           # q_h0,q_h1,k_h0,k_h1
                ps = psum.tile([128, 512], F32, tag="ps")
                for ke in range(KE):
                    nc.tensor.matmul(
                        ps,
                        lhsT=wq_sb[:, ke, rb * 128:(rb + 1) * 128],
                        rhs=xcol[:, ke, :],
                        start=(ke == 0),
                        stop=(ke == KE - 1),
                    )
                # rope: out = t*cos + swap(t)*sin_signed, bf16 out
                dst = (q_sb if rb < HL else k_sb)[b][rb % HL]
                sl = bass.ds(soff, 512)
                tswap = ropet.tile([128, 512], F32, tag="tswap")
                tcos = ropet.tile([128, 512], F32, tag="tcos")
                nc.vector.tensor_mul(tswap[0:64, :], ps[64:128, :], sin_sb[0:64, sl])
                nc.vector.tensor_mul(tswap[64:128, :], ps[0:64, :], sin_sb[64:128, sl])
                nc.vector.tensor_mul(tcos, ps, cos_sb[:, sl])
                nc.vector.tensor_add(dst[:, sl], tcos, tswap)
            # V rows: natural layout [s, D*HL]
            for tsb in range(4):
                ps = psum.tile([128, 512], F32, tag="ps")
                vps = ps[:, 0:HL * D]
                for ke in range(KE):
                    nc.tensor.matmul(
                        vps,
                        lhsT=xcol[:, ke, tsb * 128:(tsb + 1) * 128],
                        rhs=wq_sb[:, ke, 2 * HL * 128:],
                        start=(ke == 0),
                        stop=(ke == KE - 1),
                    )
                blk = (soff // 128) + tsb
                nc.vector.tensor_copy(v_sb[b][:, blk, :], vps)

        # ================= phase 2: attention =================
        for b in range(B):
            for hl in range(HL):
                for p in range(NPANEL):
                    nkb = 4 * p + 4
                    attn_ts = []
                    for kb in range(nkb):
                        qoff = max(0, kb - 4 * p) * 128
                        at = attnp.tile([128, 512], BF16, tag="attn")
                        attn_ts.append(at)
                        if qoff > 0:
                            nc.vector.memset(at[:, 0:qoff], 0.0)
                        ps = psum.tile([128, 512], F32, tag="ps")
                        nc.tensor.matmul(
                            ps[:, 0:512 - qoff],
                            lhsT=k_sb[b][hl][:, kb * 128:(kb + 1) * 128],
                            rhs=q_sb[b][hl][:, p * 512 + qoff:(p + 1) * 512],
                            start=True,
                            stop=True,
                        )
                        nc.scalar.activation(
                            at[:, qoff:512],
                            ps[:, 0:512 - qoff],
                            mybir.ActivationFunctionType.Exp,
                            scale=SOFTMAX_SCALE,
                        )
                        if kb >= 4 * p:  # diagonal block: zero k>q half
                            nc.vector.tensor_mul(
                                at[:, qoff:qoff + 128], at[:, qoff:qoff + 128], mask_sb
                            )
                    # A @ V (accumulate over k-blocks) -> yT [D, 512]
                    yps = psum.tile([128, 512], F32, tag="ps")
                    for kb in range(nkb):
                        nc.tensor.matmul(
                            yps,
                            lhsT=v_sb[b][:, kb, hl * D:(hl + 1) * D],
                            rhs=attn_ts[kb],
                            start=(kb == 0),
                            stop=(kb == nkb - 1),
                        )
                    # softmax sums over k via ones-matmul
                    sps = psum.tile([1, 512], F32, tag="sps", bufs=2)
                    for kb in range(nkb):
                        nc.tensor.matmul(
                            sps,
                            lhsT=ones_k,
                            rhs=attn_ts[kb],
                            start=(kb == 0),
                            stop=(kb == nkb - 1),
                        )
                    rrow = rowp.tile([1, 512], F32, tag="rrow")
                    nc.vector.reciprocal(rrow, sps)
                    bps = psum.tile([128, 512], F32, tag="ps")
                    nc.tensor.matmul(bps, lhsT=ones_1, rhs=rrow, start=True, stop=True)
                    rb_sb = rowp.tile([128, 512], F32, tag="rbsb")
                    nc.scalar.copy(rb_sb, bps)
                    nc.vector.tensor_mul(y_sb[b][hl][:, p * 512:(p + 1) * 512], yps, rb_sb)

        # ================= phase 3: out projection (partial) =================
        for b in range(B):
            for tkb in range(NB):
                tok0 = b * S + tkb * 128
                for oc in range(E // 512):
                    ops = psum.tile([128, 512], F32, tag="ps")
                    for hl in range(HL):
                        nc.tensor.matmul(
                            ops,
                            lhsT=y_sb[b][hl][:, tkb * 128:(tkb + 1) * 128],
                            rhs=wo_sb[:, hl, oc * 512:(oc + 1) * 512],
                            start=(hl == 0),
                            stop=(hl == HL - 1),
                        )
                    ot = outp.tile([128, 512], F32, tag="ot")
                    nc.scalar.copy(ot, ops)
                    nc.sync.dma_start(
                        out=out[tok0:tok0 + 128, oc * 512:(oc + 1) * 512], in_=ot
                    )


def build():
    nc = bacc.Bacc("TRN2", target_bir_lowering=False, debug=False)
    xT = nc.dram_tensor("xT", [E, NTOK], BF16, kind="ExternalInput").ap()
    wqkvT = nc.dram_tensor("wqkvT", [E, 3 * HL * D], BF16, kind="ExternalInput").ap()
    w_outT = nc.dram_tensor("w_outT", [HL * D, E], BF16, kind="ExternalInput").ap()
    out = nc.dram_tensor("out", [NTOK, E], F32, kind="ExternalOutput").ap()

    cosT, sinS = _rope_tables()
    cos_d = nc.inline_tensor(cosT, name="cos_t").ap()
    sin_d = nc.inline_tensor(sinS, name="sin_t").ap()
    # maskT01[k, q] = 1 where k <= q (valid), else 0 — transposed-causal
    mask = np.triu(np.ones((128, 128), np.float32)).astype(ml_dtypes.bfloat16)
    mask_d = nc.inline_tensor(mask, name="maskT01").ap()

    with tile.TileContext(nc) as tc:
        _emit(nc, tc, xT, wqkvT, w_outT, out, cos_d, sin_d, mask_d)
    nc.compile()
    return nc


def make_in_maps(x, w_qkv, w_out):
    bf = ml_dtypes.bfloat16
    x2 = np.asarray(x, np.float32).reshape(NTOK, E)
    xT = np.ascontiguousarray(x2.astype(bf).T)                      # [E, NTOK]
    w_qkv = np.asarray(w_qkv, np.float32)
    w_out = np.asarray(w_out, np.float32)
    in_maps = []
    for c in range(NCORES):
        hs = [HL * c + j for j in range(HL)]
        rows = np.concatenate(
            [w_qkv[t * E + h * D:t * E + (h + 1) * D] for t in range(3) for h in hs]
        )                                                           # [768, E]
        wqkvT = np.ascontiguousarray(rows.astype(bf).T)             # [E, 768]
        w_outT = np.ascontiguousarray(
            w_out[:, c * HL * D:(c + 1) * HL * D].astype(bf).T      # [256, E]
        )
        in_maps.append({"xT": xT, "wqkvT": wqkvT, "w_outT": w_outT})
    return in_maps


_NC = None


def kernel(x, w_qkv, w_out):
    global _NC
    if _NC is None:
        _NC = build()
    in_maps = make_in_maps(x, w_qkv, w_out)
    res = run_bass_kernel_spmd(_NC, in_maps, core_ids=list(range(NCORES)))
    total = np.zeros((NTOK, E), np.float32)
    for r in res.results:
        total += r["out"]
    return total.reshape(B, S, E)


# revision 8
# speedup vs baseline: 1.2499x; 1.2499x over previous
"""Causal self-attention (B=2, S=2048, E=2048, H=16, rope) on 8 TRN2 NeuronCores.

Sharding: tensor-parallel over heads. Each core owns 2 heads (both batches):
w_qkv rows / w_out columns for its heads; every core reads the full x
(replicated, bf16, pre-transposed) and produces a partial [B*S, E] f32
output; the host sums the 8 partials (the "all-reduce").

Per-core kernel:
  - xT [E, B*S] bf16 serves as matmul rhs (Q/K projections -> QT/KT arrive
    transposed [D, S], the layout attention wants) and as lhsT (V
    projection, natural [S, D]).
  - scores are computed transposed: scoresT[k,q] = KT^T @ QT, in panels of
    512 q columns. exp runs on ScalarE (softmax scale folded into the
    activation scale); causal masking = per-kb column offsets + one bf16
    0/1 mask multiply on the diagonal block; the invalid prefix of each
    attn tile is memset to 0 so A@V / sums can run full-width.
  - softmax sums over k (partition dim) use a ones[128,128] matmul that
    produces the column sums already broadcast across all 128 partitions;
    reciprocal + multiply fold normalization into the y^T PSUM evacuation.
  - attn^T feeds A@V as lhsT directly - no transposes anywhere.
  - rope is applied on DVE during QKV-PSUM evacuation with [D, S] cos /
    signed-sin tables; the half-rotation uses a partition-rolled sin table
    so both multiplies are full-width.
"""

import math

import numpy as np
import ml_dtypes

import concourse.bass as bass
import concourse.mybir as mybir
import concourse.tile as tile
from concourse import bacc
from concourse.bass_utils import run_bass_kernel_spmd

B, S, E, H, D = 2, 2048, 2048, 16, 128
NCORES = 8
HL = H // NCORES            # heads per core = 2
NTOK = B * S                # 4096
KE = E // 128               # 16 contraction chunks
NB = S // 128               # 16 k/token blocks per batch
NPANEL = S // 512           # 4 q panels per batch
SOFTMAX_SCALE = 1.0 / math.sqrt(D)
BF16 = mybir.dt.bfloat16
F32 = mybir.dt.float32

ROPE_BASE = 10000.0


def _rope_tables():
    inv_freq = 1.0 / (ROPE_BASE ** (np.arange(0, D, 2, dtype=np.float32) / D))
    pos = np.arange(S, dtype=np.float32)
    freqs = np.outer(pos, inv_freq)               # [S, D/2]
    emb = np.concatenate([freqs, freqs], -1)      # [S, D]
    cosT = np.cos(emb).T.astype(np.float32)       # [D, S]
    sinT = np.sin(emb).T.astype(np.float32)
    sinS = sinT.copy()
    sinS[: D // 2] *= -1.0                        # signed: rotate_half sign folded in
    return np.ascontiguousarray(cosT), np.ascontiguousarray(sinS)


def _attn_panel(nc, pools, b, hl, p, q_sb, k_sb, v_sb, y_sb, mask_sb, ones_kk):
    attnp, psum, evacp = pools
    nkb = 4 * p + 4
    yps = psum.tile([128, 512], F32, tag="yps", bufs=2, name=f"yps{b}{hl}{p}")
    sps = psum.tile([128, 512], F32, tag="sps", bufs=2, name=f"sps{b}{hl}{p}")
    for kb in range(nkb):
        qoff = max(0, kb - 4 * p) * 128
        at = attnp.tile([128, 512], BF16, tag="attn", name=f"at{b}{hl}{p}{kb}")
        if qoff > 0:
            nc.vector.memset(at[:, 0:qoff], 0.0)
        ps = psum.tile([128, 512], F32, tag="ps", name=f"sc{b}{hl}{p}{kb}")
        nc.tensor.matmul(
            ps[:, 0:512 - qoff],
            lhsT=k_sb[b][hl][:, kb * 128:(kb + 1) * 128],
            rhs=q_sb[b][hl][:, p * 512 + qoff:(p + 1) * 512],
            start=True,
            stop=True,
        )
        nc.scalar.activation(
            at[:, qoff:512],
            ps[:, 0:512 - qoff],
            mybir.ActivationFunctionType.Exp,
            scale=SOFTMAX_SCALE,
        )
        if kb >= 4 * p:  # diagonal block: zero the k>q half
            nc.vector.tensor_mul(
                at[:, qoff:qoff + 128], at[:, qoff:qoff + 128], mask_sb
            )
        nc.tensor.matmul(
            yps,
            lhsT=v_sb[b][:, kb, hl * D:(hl + 1) * D],
            rhs=at,
            start=(kb == 0),
            stop=(kb == nkb - 1),
        )
        nc.tensor.matmul(
            sps,
            lhsT=ones_kk,
            rhs=at,
            start=(kb == 0),
            stop=(kb == nkb - 1),
        )
    rb_sb = evacp.tile([128, 512], F32, tag="rb", name=f"rb{b}{hl}{p}")
    nc.vector.reciprocal(rb_sb, sps)
    nc.vector.tensor_mul(y_sb[b][hl][:, p * 512:(p + 1) * 512], yps, rb_sb)


def _emit(nc, tc, xT, wqkvT, w_outT, out, cos_d, sin_d, mask_d):
    from contextlib import ExitStack

    ctx = ExitStack()
    with ctx:
        singles = ctx.enter_context(tc.tile_pool(name="singles", bufs=1))
        xpool = ctx.enter_context(tc.tile_pool(name="xcol", bufs=2))
        persist = ctx.enter_context(tc.tile_pool(name="persist", bufs=1))
        ropet = ctx.enter_context(tc.tile_pool(name="ropet", bufs=3))
        attnp = ctx.enter_context(tc.tile_pool(name="attn", bufs=8))
        evacp = ctx.enter_context(tc.tile_pool(name="evac", bufs=2))
        outp = ctx.enter_context(tc.tile_pool(name="outp", bufs=4))
        psum = ctx.enter_context(tc.tile_pool(name="psum", bufs=2, space="PSUM"))

        # ---- constants ----
        wq_sb = []
        for ke in range(KE):
            w = singles.tile([128, 3 * HL * D], BF16, tag=f"wq{ke}", name=f"wq{ke}")
            nc.sync.dma_start(out=w, in_=wqkvT[ke * 128:(ke + 1) * 128, :])
            wq_sb.append(w)
        wo_sb = singles.tile([128, HL, E], BF16, tag="wo")
        for hl in range(HL):
            nc.sync.dma_start(out=wo_sb[:, hl, :], in_=w_outT[hl * 128:(hl + 1) * 128, :])
        cos_sb = singles.tile([128, S], F32, tag="cos")
        sin_sb = singles.tile([128, S], F32, tag="sin")
        nc.sync.dma_start(out=cos_sb, in_=cos_d)
        nc.sync.dma_start(out=sin_sb, in_=sin_d)
        mask_sb = singles.tile([128, 128], BF16, tag="mask")
        nc.sync.dma_start(out=mask_sb, in_=mask_d)
        ones_kk = singles.tile([128, 128], BF16, tag="oneskk")
        nc.vector.memset(ones_kk, 1.0)

        # ---- persistent per-(b,h) tensors ----
        q_sb = [[persist.tile([128, S], BF16, tag=f"q{b}{h}", name=f"q{b}{h}") for h in range(HL)] for b in range(B)]
        k_sb = [[persist.tile([128, S], BF16, tag=f"k{b}{h}", name=f"k{b}{h}") for h in range(HL)] for b in range(B)]
        v_sb = [persist.tile([128, NB, HL * D], BF16, tag=f"v{b}", name=f"v{b}") for b in range(B)]
        y_sb = [[persist.tile([128, S], BF16, tag=f"y{b}{h}", name=f"y{b}{h}") for h in range(HL)] for b in range(B)]

        pools = (attnp, psum, evacp)

        for b in range(B):
            # ---- QKV projection + rope for batch b ----
            for sb4 in range(S // 512):        # 4 column-blocks of 512 tokens
                tb = b * (S // 512) + sb4
                soff = sb4 * 512
                xc = []
                for ke in range(KE):
                    x1 = xpool.tile([128, 512], BF16, tag=f"xc{ke}", name=f"xc{tb}_{ke}")
                    nc.sync.dma_start(
                        out=x1,
                        in_=xT[ke * 128:(ke + 1) * 128, tb * 512:(tb + 1) * 512],
                    )
                    xc.append(x1)
                # Q/K rows: transposed layout [D, s]
                for rb in range(2 * HL):       # q_h0,q_h1,k_h0,k_h1
                    ps = psum.tile([128, 512], F32, tag="ps", name=f"qk{tb}{rb}")
                    for ke in range(KE):
                        nc.tensor.matmul(
                            ps,
                            lhsT=wq_sb[ke][:, rb * 128:(rb + 1) * 128],
                            rhs=xc[ke],
                            start=(ke == 0),
                            stop=(ke == KE - 1),
                        )
                    # rope: dst = ps*cos + swap(ps*sinRoll)
                    dst = (q_sb if rb < HL else k_sb)[b][rb % HL]
                    sl = bass.ds(soff, 512)
                    tsw = ropet.tile([128, 512], F32, tag="tsw", name=f"tsw{tb}{rb}")
                    tco = ropet.tile([128, 512], F32, tag="tco", name=f"tco{tb}{rb}")
                    nc.vector.tensor_mul(tsw[0:64, :], ps[64:128, :], sin_sb[0:64, sl])
                    nc.vector.tensor_mul(tsw[64:128, :], ps[0:64, :], sin_sb[64:128, sl])
                    nc.vector.tensor_mul(tco, ps, cos_sb[:, sl])
                    nc.vector.tensor_add(dst[:, sl], tco, tsw)
                # V rows: natural layout [s, D*HL]
                for tsb in range(4):
                    ps = psum.tile([128, 512], F32, tag="ps", name=f"v{tb}{tsb}")
                    vps = ps[:, 0:HL * D]
                    for ke in range(KE):
                        nc.tensor.matmul(
                            vps,
                            lhsT=xc[ke][:, tsb * 128:(tsb + 1) * 128],
                            rhs=wq_sb[ke][:, 2 * HL * 128:],
                            start=(ke == 0),
                            stop=(ke == KE - 1),
                        )
                    blk = (soff // 128) + tsb
                    nc.vector.tensor_copy(v_sb[b][:, blk, :], vps)

            # ---- attention + out-projection for batch b, panel by panel ----
            for p in range(NPANEL):
                for hl in range(HL):
                    _attn_panel(nc, pools, b, hl, p, q_sb, k_sb, v_sb, y_sb,
                                mask_sb, ones_kk)
                for tkb in range(4 * p, 4 * p + 4):
                    tok0 = b * S + tkb * 128
                    for oc in range(E // 512):
                        ops = psum.tile([128, 512], F32, tag="ops", name=f"o{b}{tkb}{oc}")
                        for hl in range(HL):
                            nc.tensor.matmul(
                                ops,
                                lhsT=y_sb[b][hl][:, tkb * 128:(tkb + 1) * 128],
                                rhs=wo_sb[:, hl, oc * 512:(oc + 1) * 512],
                                start=(hl == 0),
                                stop=(hl == HL - 1),
                            )
                        ot = outp.tile([128, 512], F32, tag="ot", name=f"ot{b}{tkb}{oc}")
                        nc.scalar.copy(ot, ops)
                        nc.sync.dma_start(
                            out=out[tok0:tok0 + 128, oc * 512:(oc + 1) * 512], in_=ot
                        )


def build():
    nc = bacc.Bacc("TRN2", target_bir_lowering=False, debug=False)
    xT = nc.dram_tensor("xT", [E, NTOK], BF16, kind="ExternalInput").ap()
    wqkvT = nc.dram_tensor("wqkvT", [E, 3 * HL * D], BF16, kind="ExternalInput").ap()
    w_outT = nc.dram_tensor("w_outT", [HL * D, E], BF16, kind="ExternalInput").ap()
    out = nc.dram_tensor("out", [NTOK, E], F32, kind="ExternalOutput").ap()

    cosT, sinRoll = _rope_tables()
    cos_d = nc.inline_tensor(cosT, name="cos_t").ap()
    sin_d = nc.inline_tensor(sinRoll, name="sin_t").ap()
    # maskT01[k, q] = 1 where k <= q (valid), else 0 — transposed-causal
    mask = np.triu(np.ones((128, 128), np.float32)).astype(ml_dtypes.bfloat16)
    mask_d = nc.inline_tensor(mask, name="maskT01").ap()

    with tile.TileContext(nc) as tc:
        _emit(nc, tc, xT, wqkvT, w_outT, out, cos_d, sin_d, mask_d)
    nc.compile()
    return nc


def make_in_maps(x, w_qkv, w_out):
    bf = ml_dtypes.bfloat16
    x2 = np.asarray(x, np.float32).reshape(NTOK, E)
    xT = np.ascontiguousarray(x2.astype(bf).T)                      # [E, NTOK]
    w_qkv = np.asarray(w_qkv, np.float32)
    w_out = np.asarray(w_out, np.float32)
    in_maps = []
    for c in range(NCORES):
        hs = [HL * c + j for j in range(HL)]
        rows = np.concatenate(
            [w_qkv[t * E + h * D:t * E + (h + 1) * D] for t in range(3) for h in hs]
        )                                                           # [768, E]
        wqkvT = np.ascontiguousarray(rows.astype(bf).T)             # [E, 768]
        w_outT = np.ascontiguousarray(
            w_out[:, c * HL * D:(c + 1) * HL * D].astype(bf).T      # [256, E]
        )
        in_maps.append({"xT": xT, "wqkvT": wqkvT, "w_outT": w_outT})
    return in_maps


_NC = None


def kernel(x, w_qkv, w_out):
    global _NC
    if _NC is None:
        _NC = build()
    in_maps = make_in_maps(x, w_qkv, w_out)
    res = run_bass_kernel_spmd(_NC, in_maps, core_ids=list(range(NCORES)))
    total = np.zeros((NTOK, E), np.float32)
    for r in res.results:
        total += r["out"]
    return total.reshape(B, S, E)


# revision 9
# speedup vs baseline: 1.3551x; 1.0842x over previous
"""Causal self-attention (B=2, S=2048, E=2048, H=16, rope) on 8 TRN2 NeuronCores.

Sharding: tensor-parallel over heads. Each core owns 2 heads (both batches):
w_qkv rows / w_out columns for its heads; every core reads the full x
(replicated, bf16, pre-transposed) and produces a partial [B*S, E] f32
output; the host sums the 8 partials (the "all-reduce").

Per-core kernel:
  - xT [E, B*S] bf16 serves as matmul rhs (Q/K projections -> QT/KT arrive
    transposed [D, S], the layout attention wants) and as lhsT (V
    projection, natural [S, D]).
  - scores are computed transposed: scoresT[k,q] = KT^T @ QT, in panels of
    512 q columns. exp runs on ScalarE (softmax scale folded into the
    activation scale); causal masking = per-kb column offsets + one bf16
    0/1 mask multiply on the diagonal block; the invalid prefix of each
    attn tile is memset to 0 so A@V / sums can run full-width.
  - softmax sums over k (partition dim) use a ones[128,128] matmul that
    produces the column sums already broadcast across all 128 partitions;
    reciprocal + multiply fold normalization into the y^T PSUM evacuation.
  - attn^T feeds A@V as lhsT directly - no transposes anywhere.
  - rope is applied on DVE during QKV-PSUM evacuation with [D, S] cos /
    signed-sin tables; the half-rotation uses a partition-rolled sin table
    so both multiplies are full-width.
"""

import math

import numpy as np
import ml_dtypes

import concourse.bass as bass
import concourse.mybir as mybir
import concourse.tile as tile
from concourse import bacc
from concourse.bass_utils import run_bass_kernel_spmd

B, S, E, H, D = 2, 2048, 2048, 16, 128
NCORES = 8
HL = H // NCORES            # heads per core = 2
NTOK = B * S                # 4096
KE = E // 128               # 16 contraction chunks
NB = S // 128               # 16 k/token blocks per batch
NPANEL = S // 512           # 4 q panels per batch
SOFTMAX_SCALE = 1.0 / math.sqrt(D)
BF16 = mybir.dt.bfloat16
F32 = mybir.dt.float32

ROPE_BASE = 10000.0


def _rope_tables():
    inv_freq = 1.0 / (ROPE_BASE ** (np.arange(0, D, 2, dtype=np.float32) / D))
    pos = np.arange(S, dtype=np.float32)
    freqs = np.outer(pos, inv_freq)               # [S, D/2]
    emb = np.concatenate([freqs, freqs], -1)      # [S, D]
    cosT = np.cos(emb).T.astype(np.float32)       # [D, S]
    sinT = np.sin(emb).T.astype(np.float32)
    sinS = sinT.copy()
    sinS[: D // 2] *= -1.0                        # signed: rotate_half sign folded in
    return np.ascontiguousarray(cosT), np.ascontiguousarray(sinS)


def _attn_panel(nc, pools, b, hl, p, q_sb, k_sb, v_sb, y_sb, mask_sb, ones_kk):
    attnp, psum, evacp = pools
    nkb = 4 * p + 4
    yps = psum.tile([128, 512], F32, tag="yps", bufs=2, name=f"yps{b}{hl}{p}")
    sps = psum.tile([128, 512], F32, tag="sps", bufs=2, name=f"sps{b}{hl}{p}")
    for kb in range(nkb):
        qoff = max(0, kb - 4 * p) * 128
        at = attnp.tile([128, 512], BF16, tag="attn", name=f"at{b}{hl}{p}{kb}")
        if qoff > 0:
            nc.vector.memset(at[:, 0:qoff], 0.0)
        ps = psum.tile([128, 512], F32, tag="ps", name=f"sc{b}{hl}{p}{kb}")
        nc.tensor.matmul(
            ps[:, 0:512 - qoff],
            lhsT=k_sb[b][hl][:, kb * 128:(kb + 1) * 128],
            rhs=q_sb[b][hl][:, p * 512 + qoff:(p + 1) * 512],
            start=True,
            stop=True,
        )
        nc.scalar.activation(
            at[:, qoff:512],
            ps[:, 0:512 - qoff],
            mybir.ActivationFunctionType.Exp,
            scale=SOFTMAX_SCALE,
        )
        if kb >= 4 * p:  # diagonal block: zero the k>q half
            nc.vector.tensor_mul(
                at[:, qoff:qoff + 128], at[:, qoff:qoff + 128], mask_sb
            )
        nc.tensor.matmul(
            yps,
            lhsT=v_sb[b][:, kb, hl * D:(hl + 1) * D],
            rhs=at,
            start=(kb == 0),
            stop=(kb == nkb - 1),
        )
        nc.tensor.matmul(
            sps,
            lhsT=ones_kk,
            rhs=at,
            start=(kb == 0),
            stop=(kb == nkb - 1),
        )
    rb_sb = evacp.tile([128, 512], F32, tag="rb", name=f"rb{b}{hl}{p}")
    nc.vector.reciprocal_approx_fast(out=rb_sb, in_=sps)
    nc.vector.tensor_mul(y_sb[b][hl][:, p * 512:(p + 1) * 512], yps, rb_sb)


def _emit(nc, tc, xT, wqkvT, w_outT, out, cos_d, sin_d, mask_d):
    from contextlib import ExitStack

    ctx = ExitStack()
    with ctx:
        singles = ctx.enter_context(tc.tile_pool(name="singles", bufs=1))
        xpool = ctx.enter_context(tc.tile_pool(name="xcol", bufs=2))
        persist = ctx.enter_context(tc.tile_pool(name="persist", bufs=1))
        ropet = ctx.enter_context(tc.tile_pool(name="ropet", bufs=3))
        attnp = ctx.enter_context(tc.tile_pool(name="attn", bufs=8))
        evacp = ctx.enter_context(tc.tile_pool(name="evac", bufs=2))
        outp = ctx.enter_context(tc.tile_pool(name="outp", bufs=4))
        psum = ctx.enter_context(tc.tile_pool(name="psum", bufs=2, space="PSUM"))

        # ---- constants ----
        wq_sb = []
        for ke in range(KE):
            w = singles.tile([128, 3 * HL * D], BF16, tag=f"wq{ke}", name=f"wq{ke}")
            nc.sync.dma_start(out=w, in_=wqkvT[ke * 128:(ke + 1) * 128, :])
            wq_sb.append(w)
        wo_sb = singles.tile([128, HL, E], BF16, tag="wo")
        for hl in range(HL):
            nc.sync.dma_start(out=wo_sb[:, hl, :], in_=w_outT[hl * 128:(hl + 1) * 128, :])
        cos_sb = singles.tile([128, S], F32, tag="cos")
        sin_sb = singles.tile([128, S], F32, tag="sin")
        nc.sync.dma_start(out=cos_sb, in_=cos_d)
        nc.sync.dma_start(out=sin_sb, in_=sin_d)
        mask_sb = singles.tile([128, 128], BF16, tag="mask")
        nc.sync.dma_start(out=mask_sb, in_=mask_d)
        ones_kk = singles.tile([128, 128], BF16, tag="oneskk")
        nc.vector.memset(ones_kk, 1.0)

        # ---- persistent per-(b,h) tensors ----
        q_sb = [[persist.tile([128, S], BF16, tag=f"q{b}{h}", name=f"q{b}{h}") for h in range(HL)] for b in range(B)]
        k_sb = [[persist.tile([128, S], BF16, tag=f"k{b}{h}", name=f"k{b}{h}") for h in range(HL)] for b in range(B)]
        v_sb = [persist.tile([128, NB, HL * D], BF16, tag=f"v{b}", name=f"v{b}") for b in range(B)]
        y_sb = [[persist.tile([128, S], BF16, tag=f"y{b}{h}", name=f"y{b}{h}") for h in range(HL)] for b in range(B)]

        pools = (attnp, psum, evacp)

        for b in range(B):
            # ---- QKV projection + rope for batch b ----
            for sb4 in range(S // 512):        # 4 column-blocks of 512 tokens
                tb = b * (S // 512) + sb4
                soff = sb4 * 512
                xc = []
                for ke in range(KE):
                    x1 = xpool.tile([128, 512], BF16, tag=f"xc{ke}", name=f"xc{tb}_{ke}")
                    nc.sync.dma_start(
                        out=x1,
                        in_=xT[ke * 128:(ke + 1) * 128, tb * 512:(tb + 1) * 512],
                    )
                    xc.append(x1)
                # Q/K rows: transposed layout [D, s]
                for rb in range(2 * HL):       # q_h0,q_h1,k_h0,k_h1
                    ps = psum.tile([128, 512], F32, tag="ps", name=f"qk{tb}{rb}")
                    for ke in range(KE):
                        nc.tensor.matmul(
                            ps,
                            lhsT=wq_sb[ke][:, rb * 128:(rb + 1) * 128],
                            rhs=xc[ke],
                            start=(ke == 0),
                            stop=(ke == KE - 1),
                        )
                    # rope: dst = ps*cos + swap(ps*sinRoll)
                    dst = (q_sb if rb < HL else k_sb)[b][rb % HL]
                    sl = bass.ds(soff, 512)
                    tsw = ropet.tile([128, 512], F32, tag="tsw", name=f"tsw{tb}{rb}")
                    tco = ropet.tile([128, 512], F32, tag="tco", name=f"tco{tb}{rb}")
                    nc.vector.tensor_mul(tsw[0:64, :], ps[64:128, :], sin_sb[0:64, sl])
                    nc.vector.tensor_mul(tsw[64:128, :], ps[0:64, :], sin_sb[64:128, sl])
                    nc.vector.tensor_mul(tco, ps, cos_sb[:, sl])
                    nc.vector.tensor_add(dst[:, sl], tco, tsw)
                # V rows: natural layout [s, D*HL]
                for tsb in range(4):
                    ps = psum.tile([128, 512], F32, tag="ps", name=f"v{tb}{tsb}")
                    vps = ps[:, 0:HL * D]
                    for ke in range(KE):
                        nc.tensor.matmul(
                            vps,
                            lhsT=xc[ke][:, tsb * 128:(tsb + 1) * 128],
                            rhs=wq_sb[ke][:, 2 * HL * 128:],
                            start=(ke == 0),
                            stop=(ke == KE - 1),
                        )
                    blk = (soff // 128) + tsb
                    nc.vector.tensor_copy(v_sb[b][:, blk, :], vps)

            # ---- attention + out-projection for batch b, panel by panel ----
            for p in range(NPANEL):
                for hl in range(HL):
                    _attn_panel(nc, pools, b, hl, p, q_sb, k_sb, v_sb, y_sb,
                                mask_sb, ones_kk)
                for tkb in range(4 * p, 4 * p + 4):
                    tok0 = b * S + tkb * 128
                    for oc in range(E // 512):
                        ops = psum.tile([128, 512], F32, tag="ops", name=f"o{b}{tkb}{oc}")
                        for hl in range(HL):
                            nc.tensor.matmul(
                                ops,
                                lhsT=y_sb[b][hl][:, tkb * 128:(tkb + 1) * 128],
                                rhs=wo_sb[:, hl, oc * 512:(oc + 1) * 512],
                                start=(hl == 0),
                                stop=(hl == HL - 1),
                            )
                        ot = outp.tile([128, 512], F32, tag="ot", name=f"ot{b}{tkb}{oc}")
                        nc.scalar.copy(ot, ops)
                        nc.sync.dma_start(
                            out=out[tok0:tok0 + 128, oc * 512:(oc + 1) * 512], in_=ot
                        )


def build():
    nc = bacc.Bacc("TRN2", target_bir_lowering=False, debug=False)
    xT = nc.dram_tensor("xT", [E, NTOK], BF16, kind="ExternalInput").ap()
    wqkvT = nc.dram_tensor("wqkvT", [E, 3 * HL * D], BF16, kind="ExternalInput").ap()
    w_outT = nc.dram_tensor("w_outT", [HL * D, E], BF16, kind="ExternalInput").ap()
    out = nc.dram_tensor("out", [NTOK, E], F32, kind="ExternalOutput").ap()

    cosT, sinRoll = _rope_tables()
    cos_d = nc.inline_tensor(cosT, name="cos_t").ap()
    sin_d = nc.inline_tensor(sinRoll, name="sin_t").ap()
    # maskT01[k, q] = 1 where k <= q (valid), else 0 — transposed-causal
    mask = np.triu(np.ones((128, 128), np.float32)).astype(ml_dtypes.bfloat16)
    mask_d = nc.inline_tensor(mask, name="maskT01").ap()

    with tile.TileContext(nc) as tc:
        _emit(nc, tc, xT, wqkvT, w_outT, out, cos_d, sin_d, mask_d)
    nc.compile()
    return nc


def make_in_maps(x, w_qkv, w_out):
    bf = ml_dtypes.bfloat16
    x2 = np.asarray(x, np.float32).reshape(NTOK, E)
    xT = np.ascontiguousarray(x2.astype(bf).T)                      # [E, NTOK]
    w_qkv = np.asarray(w_qkv, np.float32)
    w_out = np.asarray(w_out, np.float32)
    in_maps = []
    for c in range(NCORES):
        hs = [HL * c + j for j in range(HL)]
        rows = np.concatenate(
            [w_qkv[t * E + h * D:t * E + (h + 1) * D] for t in range(3) for h in hs]
        )                                                           # [768, E]
        wqkvT = np.ascontiguousarray(rows.astype(bf).T)             # [E, 768]
        w_outT = np.ascontiguousarray(
            w_out[:, c * HL * D:(c + 1) * HL * D].astype(bf).T      # [256, E]
        )
        in_maps.append({"xT": xT, "wqkvT": wqkvT, "w_outT": w_outT})
    return in_maps


_NC = None


def kernel(x, w_qkv, w_out):
    global _NC
    if _NC is None:
        _NC = build()
    in_maps = make_in_maps(x, w_qkv, w_out)
    res = run_bass_kernel_spmd(_NC, in_maps, core_ids=list(range(NCORES)))
    total = np.zeros((NTOK, E), np.float32)
    for r in res.results:
        total += r["out"]
    return total.reshape(B, S, E)


# revision 11
# speedup vs baseline: 1.4797x; 1.0919x over previous
"""Causal self-attention (B=2, S=2048, E=2048, H=16, rope) on 8 TRN2 NeuronCores.

Sharding: tensor-parallel over heads. Each core owns 2 heads (both batches):
w_qkv rows / w_out columns for its heads; every core reads the full x
(replicated, bf16, pre-transposed) and produces a partial [B*S, E] f32
output; the host sums the 8 partials (the "all-reduce").

Per-core kernel:
  - xT [E, B*S] bf16 serves as matmul rhs (Q/K projections -> QT/KT arrive
    transposed [D, S], the layout attention wants) and as lhsT (V
    projection, natural [S, D]).
  - scores are computed transposed: scoresT[k,q] = KT^T @ QT, in panels of
    512 q columns. exp runs on ScalarE (softmax scale folded into the
    activation scale); causal masking = per-kb column offsets + one bf16
    0/1 mask multiply on the diagonal block; the invalid prefix of each
    attn tile is memset to 0 so A@V / sums can run full-width.
  - softmax sums over k (partition dim) use a ones[128,128] matmul that
    produces the column sums already broadcast across all 128 partitions;
    reciprocal + multiply fold normalization into the y^T PSUM evacuation.
  - attn^T feeds A@V as lhsT directly - no transposes anywhere.
  - rope is applied on DVE during QKV-PSUM evacuation with [D, S] cos /
    signed-sin tables; the half-rotation uses a partition-rolled sin table
    so both multiplies are full-width.
"""

import math

import numpy as np
import ml_dtypes

import concourse.bass as bass
import concourse.mybir as mybir
import concourse.tile as tile
from concourse import bacc
from concourse.bass_utils import run_bass_kernel_spmd

B, S, E, H, D = 2, 2048, 2048, 16, 128
NCORES = 8
HL = H // NCORES            # heads per core = 2
NTOK = B * S                # 4096
KE = E // 128               # 16 contraction chunks
NB = S // 128               # 16 k/token blocks per batch
NPANEL = S // 512           # 4 q panels per batch
SOFTMAX_SCALE = 1.0 / math.sqrt(D)
BF16 = mybir.dt.bfloat16
F32 = mybir.dt.float32

ROPE_BASE = 10000.0


def _rope_tables():
    inv_freq = 1.0 / (ROPE_BASE ** (np.arange(0, D, 2, dtype=np.float32) / D))
    pos = np.arange(S, dtype=np.float32)
    freqs = np.outer(pos, inv_freq)               # [S, D/2]
    emb = np.concatenate([freqs, freqs], -1)      # [S, D]
    cosT = np.cos(emb).T.astype(np.float32)       # [D, S]
    sinT = np.sin(emb).T.astype(np.float32)
    sinS = sinT.copy()
    sinS[: D // 2] *= -1.0                        # signed: rotate_half sign folded in
    return np.ascontiguousarray(cosT), np.ascontiguousarray(sinS)


def _attn_panel(nc, pools, b, hl, p, q_sb, k_sb, v_sb, y_sb, mask_sb, ones_kk):
    attnp, psum, evacp = pools
    nkb = 4 * p + 4
    yps = psum.tile([128, 512], F32, tag="yps", bufs=2, name=f"yps{b}{hl}{p}")
    sps = psum.tile([128, 512], F32, tag="sps", bufs=1, name=f"sps{b}{hl}{p}")
    for kb in range(nkb):
        qoff = max(0, kb - 4 * p) * 128
        at = attnp.tile([128, 512], BF16, tag="attn", name=f"at{b}{hl}{p}{kb}")
        if qoff > 0:
            nc.vector.memset(at[:, 0:qoff], 0.0)
        ps = psum.tile([128, 512], F32, tag="ps", bufs=3, name=f"sc{b}{hl}{p}{kb}")
        nc.tensor.matmul(
            ps[:, 0:512 - qoff],
            lhsT=k_sb[b][hl][:, kb * 128:(kb + 1) * 128],
            rhs=q_sb[b][hl][:, p * 512 + qoff:(p + 1) * 512],
            start=True,
            stop=True,
        )
        nc.scalar.activation(
            at[:, qoff:512],
            ps[:, 0:512 - qoff],
            mybir.ActivationFunctionType.Exp,
            scale=SOFTMAX_SCALE,
        )
        if kb >= 4 * p:  # diagonal block: zero the k>q half
            nc.vector.tensor_mul(
                at[:, qoff:qoff + 128], at[:, qoff:qoff + 128], mask_sb
            )
        nc.tensor.matmul(
            yps,
            lhsT=v_sb[b][:, kb, hl * D:(hl + 1) * D],
            rhs=at,
            start=(kb == 0),
            stop=(kb == nkb - 1),
        )
        nc.tensor.matmul(
            sps,
            lhsT=ones_kk,
            rhs=at,
            start=(kb == 0),
            stop=(kb == nkb - 1),
        )
    rb_sb = evacp.tile([128, 512], F32, tag="rb", name=f"rb{b}{hl}{p}")
    nc.vector.reciprocal_approx_fast(out=rb_sb, in_=sps)
    nc.vector.tensor_mul(y_sb[b][hl][:, p * 512:(p + 1) * 512], yps, rb_sb)


def _emit(nc, tc, xT, wqkvT, w_outT, out, cos_d, sin_d, mask_d):
    from contextlib import ExitStack

    ctx = ExitStack()
    with ctx:
        singles = ctx.enter_context(tc.tile_pool(name="singles", bufs=1))
        xpool = ctx.enter_context(tc.tile_pool(name="xcol", bufs=2))
        persist = ctx.enter_context(tc.tile_pool(name="persist", bufs=1))
        ropet = ctx.enter_context(tc.tile_pool(name="ropet", bufs=3))
        attnp = ctx.enter_context(tc.tile_pool(name="attn", bufs=8))
        evacp = ctx.enter_context(tc.tile_pool(name="evac", bufs=2))
        outp = ctx.enter_context(tc.tile_pool(name="outp", bufs=4))
        psum = ctx.enter_context(tc.tile_pool(name="psum", bufs=2, space="PSUM"))

        # ---- constants ----
        wq_sb = []
        for ke in range(KE):
            w = singles.tile([128, 3 * HL * D], BF16, tag=f"wq{ke}", name=f"wq{ke}")
            nc.sync.dma_start(out=w, in_=wqkvT[ke * 128:(ke + 1) * 128, :])
            wq_sb.append(w)
        wo_sb = singles.tile([128, HL, E], BF16, tag="wo")
        for hl in range(HL):
            nc.sync.dma_start(out=wo_sb[:, hl, :], in_=w_outT[hl * 128:(hl + 1) * 128, :])
        cos_sb = singles.tile([128, S], F32, tag="cos")
        sin_sb = singles.tile([128, S], F32, tag="sin")
        nc.sync.dma_start(out=cos_sb, in_=cos_d)
        nc.sync.dma_start(out=sin_sb, in_=sin_d)
        mask_sb = singles.tile([128, 128], BF16, tag="mask")
        nc.sync.dma_start(out=mask_sb, in_=mask_d)
        ones_kk = singles.tile([128, 128], BF16, tag="oneskk")
        nc.vector.memset(ones_kk, 1.0)

        # ---- persistent per-(b,h) tensors ----
        q_sb = [[persist.tile([128, S], BF16, tag=f"q{b}{h}", name=f"q{b}{h}") for h in range(HL)] for b in range(B)]
        k_sb = [[persist.tile([128, S], BF16, tag=f"k{b}{h}", name=f"k{b}{h}") for h in range(HL)] for b in range(B)]
        v_sb = [persist.tile([128, NB, HL * D], BF16, tag=f"v{b}", name=f"v{b}") for b in range(B)]
        y_sb = [[persist.tile([128, S], BF16, tag=f"y{b}{h}", name=f"y{b}{h}") for h in range(HL)] for b in range(B)]

        pools = (attnp, psum, evacp)

        for b in range(B):
            # ---- QKV projection + rope for batch b ----
            for sb4 in range(S // 512):        # 4 column-blocks of 512 tokens
                tb = b * (S // 512) + sb4
                soff = sb4 * 512
                xc = []
                for ke in range(KE):
                    x1 = xpool.tile([128, 512], BF16, tag=f"xc{ke}", name=f"xc{tb}_{ke}")
                    nc.sync.dma_start(
                        out=x1,
                        in_=xT[ke * 128:(ke + 1) * 128, tb * 512:(tb + 1) * 512],
                    )
                    xc.append(x1)
                # Q/K rows: transposed layout [D, s]
                for rb in range(2 * HL):       # q_h0,q_h1,k_h0,k_h1
                    ps = psum.tile([128, 512], F32, tag="ps", bufs=3, name=f"qk{tb}{rb}")
                    for ke in range(KE):
                        nc.tensor.matmul(
                            ps,
                            lhsT=wq_sb[ke][:, rb * 128:(rb + 1) * 128],
                            rhs=xc[ke],
                            start=(ke == 0),
                            stop=(ke == KE - 1),
                        )
                    # rope: dst = ps*cos + swap(ps*sinRoll)
                    dst = (q_sb if rb < HL else k_sb)[b][rb % HL]
                    sl = bass.ds(soff, 512)
                    tsw = ropet.tile([128, 512], F32, tag="tsw", name=f"tsw{tb}{rb}")
                    tco = ropet.tile([128, 512], F32, tag="tco", name=f"tco{tb}{rb}")
                    nc.vector.tensor_mul(tsw[0:64, :], ps[64:128, :], sin_sb[0:64, sl])
                    nc.vector.tensor_mul(tsw[64:128, :], ps[0:64, :], sin_sb[64:128, sl])
                    nc.vector.tensor_mul(tco, ps, cos_sb[:, sl])
                    nc.vector.tensor_add(dst[:, sl], tco, tsw)
                # V rows: natural layout [s, D*HL]
                for tsb in range(4):
                    ps = psum.tile([128, 512], F32, tag="ps", bufs=3, name=f"v{tb}{tsb}")
                    vps = ps[:, 0:HL * D]
                    for ke in range(KE):
                        nc.tensor.matmul(
                            vps,
                            lhsT=xc[ke][:, tsb * 128:(tsb + 1) * 128],
                            rhs=wq_sb[ke][:, 2 * HL * 128:],
                            start=(ke == 0),
                            stop=(ke == KE - 1),
                        )
                    blk = (soff // 128) + tsb
                    nc.vector.tensor_copy(v_sb[b][:, blk, :], vps)

            # ---- attention + out-projection for batch b, panel by panel ----
            for p in reversed(range(NPANEL)):
                for hl in range(HL):
                    _attn_panel(nc, pools, b, hl, p, q_sb, k_sb, v_sb, y_sb,
                                mask_sb, ones_kk)
                for tkb in range(4 * p, 4 * p + 4):
                    tok0 = b * S + tkb * 128
                    for oc in range(E // 512):
                        ops = psum.tile([128, 512], F32, tag="ops", name=f"o{b}{tkb}{oc}")
                        for hl in range(HL):
                            nc.tensor.matmul(
                                ops,
                                lhsT=y_sb[b][hl][:, tkb * 128:(tkb + 1) * 128],
                                rhs=wo_sb[:, hl, oc * 512:(oc + 1) * 512],
                                start=(hl == 0),
                                stop=(hl == HL - 1),
                            )
                        ot = outp.tile([128, 512], F32, tag="ot", name=f"ot{b}{tkb}{oc}")
                        if oc % 2 == 0:
                            nc.scalar.copy(ot, ops)
                        else:
                            nc.vector.tensor_copy(ot, ops)
                        nc.sync.dma_start(
                            out=out[tok0:tok0 + 128, oc * 512:(oc + 1) * 512], in_=ot
                        )


def build():
    nc = bacc.Bacc("TRN2", target_bir_lowering=False, debug=False)
    xT = nc.dram_tensor("xT", [E, NTOK], BF16, kind="ExternalInput").ap()
    wqkvT = nc.dram_tensor("wqkvT", [E, 3 * HL * D], BF16, kind="ExternalInput").ap()
    w_outT = nc.dram_tensor("w_outT", [HL * D, E], BF16, kind="ExternalInput").ap()
    out = nc.dram_tensor("out", [NTOK, E], F32, kind="ExternalOutput").ap()

    cosT, sinRoll = _rope_tables()
    cos_d = nc.inline_tensor(cosT, name="cos_t").ap()
    sin_d = nc.inline_tensor(sinRoll, name="sin_t").ap()
    # maskT01[k, q] = 1 where k <= q (valid), else 0 — transposed-causal
    mask = np.triu(np.ones((128, 128), np.float32)).astype(ml_dtypes.bfloat16)
    mask_d = nc.inline_tensor(mask, name="maskT01").ap()

    with tile.TileContext(nc) as tc:
        _emit(nc, tc, xT, wqkvT, w_outT, out, cos_d, sin_d, mask_d)
    nc.compile()
    return nc


def make_in_maps(x, w_qkv, w_out):
    bf = ml_dtypes.bfloat16
    x2 = np.asarray(x, np.float32).reshape(NTOK, E)
    xT = np.ascontiguousarray(x2.astype(bf).T)                      # [E, NTOK]
    w_qkv = np.asarray(w_qkv, np.float32)
    w_out = np.asarray(w_out, np.float32)
    in_maps = []
    for c in range(NCORES):
        hs = [HL * c + j for j in range(HL)]
        rows = np.concatenate(
            [w_qkv[t * E + h * D:t * E + (h + 1) * D] for t in range(3) for h in hs]
        )                                                           # [768, E]
        wqkvT = np.ascontiguousarray(rows.astype(bf).T)             # [E, 768]
        w_outT = np.ascontiguousarray(
            w_out[:, c * HL * D:(c + 1) * HL * D].astype(bf).T      # [256, E]
        )
        in_maps.append({"xT": xT, "wqkvT": wqkvT, "w_outT": w_outT})
    return in_maps


_NC = None


def kernel(x, w_qkv, w_out):
    global _NC
    if _NC is None:
        _NC = build()
    in_maps = make_in_maps(x, w_qkv, w_out)
    res = run_bass_kernel_spmd(_NC, in_maps, core_ids=list(range(NCORES)))
    total = np.zeros((NTOK, E), np.float32)
    for r in res.results:
        total += r["out"]
    return total.reshape(B, S, E)
